# revision 13
# baseline (speedup 1.0000x reference)
"""DMPNNConv kernel for 8 Trainium2 NeuronCores.

  h_n = relu([x ; h_e] @ W_i_w.T + W_i_b)          [N, D]
  m   = einsum('kn,nd->d', bond_n, h_n)            [D]
  h   = relu(h_n + m @ W_m_w.T + W_m_b)            [N, D]

Sharding: N (edge dim) split 8 ways; weights replicated; single [D]
all-gather (+local sum) of the message m between the two passes.

Design (vs the DRAM-scratch baseline):
  - host marshals x/h_e to feature-major fp8e4 (w1 pre-scaled x16,
    undone by the ACT scale) and bond to fp8, so the device streams
    large contiguous DMA runs with no SWDGE cast and no PE transposes.
  - z = W1.T @ [x;he] in ONE DoubleRow matmul (256-contract fp8).
  - h_n stays RESIDENT in SBUF ([128, 63488] bf16 = 124 KB/partition),
    eliminating the 32 MB/core DRAM scratch round-trip.
  - per 512-col tile: ACT relu+bias+scale -> resident hn; w broadcast
    = ones32.T@bond (PE); DVE prod = hn*wb; one batched DVE reduce per
    2048-col chunk accumulates the m partials.
  - [D] message all-gathered between passes; c = W_m m + b2 on PE.
  - pass 2: h = relu(hn + c) alternating ACT (per-partition bias) and
    DVE (stride-0 broadcast add + relu), bf16 out on two DMA queues;
    host upcasts/transposes the [D, N] output.
"""

import os
import sys

sys.path.insert(0, "/opt/trn_rl_repo")

import numpy as np

N, D, K = 500000, 128, 32
CORES = 8
T = 512                       # columns per compute tile (one PSUM bank)
NT = 124                      # tiles per core
N_SH = NT * T                 # 63488 padded tokens per core
N_PAD = CORES * N_SH          # 507904
CHUNK = 2048                  # columns per DMA chunk
NCH = N_SH // CHUNK           # 31
TPC = CHUNK // T              # tiles per chunk = 4
WSCALE = 16.0                 # fp8 weight pre-scale (undone in ACT)

USE_FP8 = os.environ.get("BASS_FP8", "1") == "1"

_cache = {}
last_results = None


def _build(nt=NT, chunk=CHUNK, fp8=None):
    import concourse.bass as bass  # noqa: F401
    import concourse.bacc as bacc
    import concourse.tile as tile
    import concourse.mybir as mybir

    n_sh = nt * T
    nch = n_sh // chunk
    tpc = chunk // T
    assert nch * chunk == n_sh
    if fp8 is None:
        fp8 = USE_FP8

    f32 = mybir.dt.float32
    bf16 = mybir.dt.bfloat16
    f8 = mybir.dt.float8e4
    in_dt = f8 if fp8 else bf16
    w1_fp8 = os.environ.get("BASS_W1DT", "bf16") == "fp8"
    w1_dt = f8 if (fp8 and w1_fp8) else bf16
    AF = mybir.ActivationFunctionType
    ALU = mybir.AluOpType
    red_mode = os.environ.get("BASS_RED", "reduce")

    nc = bacc.Bacc("TRN2", target_bir_lowering=False, debug=False,
                   num_devices=CORES)

    xT_d = nc.dram_tensor("xT", [D, n_sh], in_dt, kind="ExternalInput").ap()
    heT_d = nc.dram_tensor("heT", [D, n_sh], in_dt, kind="ExternalInput").ap()
    bond_d = nc.dram_tensor("bond", [K, n_sh], in_dt, kind="ExternalInput").ap()
    w1t_d = nc.dram_tensor("w1t", [D, 2 * D], w1_dt, kind="ExternalInput").ap()
    b1_d = nc.dram_tensor("b1", [D], f32, kind="ExternalInput").ap()
    b2_d = nc.dram_tensor("b2", [D], f32, kind="ExternalInput").ap()
    wmT_d = nc.dram_tensor("wmT", [D, D], f32, kind="ExternalInput").ap()
    h_d = nc.dram_tensor("h", [D, n_sh], bf16, kind="ExternalOutput").ap()

    with tile.TileContext(nc) as tc:
        import contextlib
        ctx = contextlib.ExitStack()
        with ctx:
            pers = ctx.enter_context(tc.tile_pool(name="pers", bufs=1))
            iox = ctx.enter_context(tc.tile_pool(name="iox", bufs=3))
            iob = ctx.enter_context(tc.tile_pool(name="iob", bufs=3))
            ioo = ctx.enter_context(tc.tile_pool(name="ioo", bufs=4))
            ps_z = ctx.enter_context(tc.tile_pool(name="ps_z", bufs=2, space="PSUM"))
            ps_w = ctx.enter_context(tc.tile_pool(name="ps_w", bufs=2, space="PSUM"))
            ps_c = ctx.enter_context(tc.tile_pool(name="ps_c", bufs=1, space="PSUM"))
            dram = ctx.enter_context(tc.tile_pool(name="dram", bufs=1, space="DRAM"))

            # ---- one-time setup -------------------------------------------
            # stationary weights: fp8 DoubleRow layout [j, 2, d] (x|he), or
            # bf16 [j, d] halves side by side.
            w1t = pers.tile([D, 2 * D], w1_dt)
            nc.sync.dma_start(w1t[:], w1t_d[:])
            b1_col = pers.tile([D, 1], f32)
            nc.sync.dma_start(b1_col[:, 0], b1_d[:])
            b2_col = pers.tile([D, 1], f32)
            nc.sync.dma_start(b2_col[:, 0], b2_d[:])
            wmt = pers.tile([D, D], f32)            # [d', d] = W_m.T
            nc.sync.dma_start(wmt[:], wmT_d[:])
            ones32 = pers.tile([K, D], in_dt)
            nc.gpsimd.memset(ones32[:], 1.0)

            hn_all = pers.tile([D, n_sh], bf16)     # resident h_n
            m_parts = pers.tile([D, nch], f32)
            junk = pers.tile([D, chunk], bf16)

            m_in = dram.tile([D], f32)
            m_gath = dram.tile([CORES * D], f32, addr_space="Shared")

            use_dr = fp8 and w1_fp8
            if use_dr:
                w1dr = w1t[:].rearrange("j (two d) -> j two d", two=2)
                dr_mode = mybir.MatmulPerfMode.DoubleRow
            zscale = 1.0 / WSCALE if use_dr else 1.0

            # ---- pass 1 ----------------------------------------------------
            for ch in range(nch):
                csl = slice(chunk * ch, chunk * (ch + 1))
                xc = iox.tile([D, 2, chunk], in_dt, tag="xhe")
                nc.sync.dma_start(xc[:, 0], xT_d[:, csl])
                nc.scalar.dma_start(xc[:, 1], heT_d[:, csl])
                bc = iob.tile([K, chunk], in_dt)
                nc.sync.dma_start(bc[:], bond_d[:, csl])

                prod = ioo.tile([D, chunk], bf16, tag="prod")
                for t in range(tpc):
                    tsl = slice(T * t, T * (t + 1))
                    gsl = slice(T * (tpc * ch + t), T * (tpc * ch + t + 1))
                    z_ps = ps_z.tile([D, T], f32, tag="z")
                    if use_dr:
                        nc.tensor.matmul(z_ps[:], w1dr, xc[:, :, tsl],
                                         start=True, stop=True,
                                         perf_mode=dr_mode)
                    else:
                        nc.tensor.matmul(z_ps[:], w1t[:, 0:D], xc[:, 0, tsl],
                                         start=True, stop=False)
                        nc.tensor.matmul(z_ps[:], w1t[:, D:2 * D],
                                         xc[:, 1, tsl],
                                         start=False, stop=True)
                    wb_ps = ps_w.tile([D, T], f32, tag="wb")
                    nc.tensor.matmul(wb_ps[:], ones32[:], bc[:, tsl],
                                     start=True, stop=True)

                    nc.scalar.activation(hn_all[:, gsl], z_ps[:], AF.Relu,
                                         bias=b1_col[:], scale=zscale)
                    nc.vector.tensor_tensor(prod[:, tsl], hn_all[:, gsl],
                                            wb_ps[:], ALU.mult)
                if red_mode == "ts":
                    nc.vector.tensor_scalar(
                        junk[:], prod[:], 1.0, 0.0, ALU.mult, ALU.add,
                        accum_out=m_parts[:, ch:ch + 1])
                else:
                    nc.vector.tensor_reduce(
                        m_parts[:, ch:ch + 1], prod[:],
                        mybir.AxisListType.X, ALU.add)

            # ---- m all-gather + local sum + c -----------------------------
            m_col = pers.tile([D, 1], f32)
            nc.vector.reduce_sum(m_col[:], m_parts[:], axis=mybir.AxisListType.X)
            nc.sync.dma_start(m_in[:], m_col[:, 0])
            nc.gpsimd.collective_compute(
                "AllGather", ALU.bypass,
                replica_groups=[list(range(CORES))],
                ins=[m_in[:].opt()], outs=[m_gath[:].opt()])
            m_g = pers.tile([D, CORES], f32)
            nc.sync.dma_start(m_g[:], m_gath[:].rearrange("(r p) -> p r", p=D))
            m_sb = pers.tile([D, 1], f32)
            nc.vector.reduce_sum(m_sb[:], m_g[:], axis=mybir.AxisListType.X)

            c_ps = ps_c.tile([D, 1], f32, tag="c")
            nc.tensor.matmul(c_ps[:], wmt[:], m_sb[:], start=True, stop=True)
            c_col = pers.tile([D, 1], f32)
            nc.vector.tensor_tensor(c_col[:], c_ps[:], b2_col[:], ALU.add)
            c_bf = pers.tile([D, 1], bf16)
            nc.vector.tensor_copy(c_bf[:], c_col[:])

            # ---- pass 2: h = relu(hn + c), bf16 out -----------------------
            c_bc = c_bf[:].broadcast_to([D, chunk])
            for ch in range(nch):
                csl = slice(chunk * ch, chunk * (ch + 1))
                ob = ioo.tile([D, chunk], bf16, tag="ob")
                if ch % 2 == 0:
                    nc.scalar.activation(ob[:], hn_all[:, csl], AF.Relu,
                                         bias=c_col[:])
                else:
                    nc.vector.tensor_tensor(ob[:], hn_all[:, csl], c_bc,
                                            ALU.add)
                    nc.vector.tensor_relu(ob[:], ob[:])
                if ch % 2 == 0:
                    nc.sync.dma_start(h_d[:, csl], ob[:])
                else:
                    nc.scalar.dma_start(h_d[:, csl], ob[:])

    nc.compile()
    return nc


def _get_nc():
    if "nc" not in _cache:
        _cache["nc"] = _build()
    return _cache["nc"]


def _ensure_ntff_hook():
    """Register the axon NTFF profile hook if the image's antenv lacks it."""
    import types
    try:
        import antenv.axon_hooks  # noqa: F401
        return
    except ImportError:
        pass
    try:
        import antenv
        from trn_agent_boot.trn_boot import _ntff_profile_via_ctypes
        mod = types.ModuleType("antenv.axon_hooks")
        _h = {"hook": None}
        mod.set_axon_ntff_profile_hook = lambda h: _h.__setitem__("hook", h)
        mod.get_axon_ntff_profile_hook = lambda: _h["hook"]
        sys.modules["antenv.axon_hooks"] = mod
        antenv.axon_hooks = mod
        hook = _ntff_profile_via_ctypes("/opt/axon/libaxon_pjrt.so")
        if hook is not None:
            mod.set_axon_ntff_profile_hook(hook)
    except Exception:
        pass


def marshal_inputs(inputs, n_sh=N_SH, fp8=None):
    """Host-side marshalling: shard + feature-major layout + dtype cast."""
    import ml_dtypes
    if fp8 is None:
        fp8 = USE_FP8
    f8np = np.dtype(ml_dtypes.float8_e4m3)
    bfnp = np.dtype(ml_dtypes.bfloat16)
    in_np = f8np if fp8 else bfnp
    w1_fp8 = os.environ.get("BASS_W1DT", "bf16") == "fp8"
    w1_np = f8np if (fp8 and w1_fp8) else bfnp
    ws = WSCALE if (fp8 and w1_fp8) else 1.0

    x = np.asarray(inputs["x"], dtype=np.float32)
    he = np.asarray(inputs["h_e"], dtype=np.float32)
    bond = np.asarray(inputs["bond_n"], dtype=np.float32)
    wi = np.asarray(inputs["W_i_w"], dtype=np.float32)
    bi = np.ascontiguousarray(np.asarray(inputs["W_i_b"], dtype=np.float32))
    wm = np.asarray(inputs["W_m_w"], dtype=np.float32)
    bm = np.ascontiguousarray(np.asarray(inputs["W_m_b"], dtype=np.float32))

    n = x.shape[0]
    w1t = np.empty((D, 2 * D), w1_np)
    if fp8 and w1_fp8:
        # DoubleRow layout [j, 2, d]: slot 0 = x half, slot 1 = h_e half
        w1t3 = w1t.reshape(D, 2, D)
        w1t3[:, 0, :] = (wi[:, 0:D].T * ws).astype(w1_np)
        w1t3[:, 1, :] = (wi[:, D:2 * D].T * ws).astype(w1_np)
    else:
        w1t[:, 0:D] = wi[:, 0:D].T.astype(w1_np)
        w1t[:, D:2 * D] = wi[:, D:2 * D].T.astype(w1_np)
    wmT = np.ascontiguousarray(wm.T)

    xT, heT = x.T, he.T
    in_maps = []
    for c in range(CORES):
        lo = c * n_sh
        hi = min(n, lo + n_sh)
        v = max(0, hi - lo)
        xc = np.zeros((D, n_sh), in_np)
        hc = np.zeros((D, n_sh), in_np)
        bc = np.zeros((K, n_sh), in_np)
        if v > 0:
            xc[:, :v] = xT[:, lo:hi]
            hc[:, :v] = heT[:, lo:hi]
            bc[:, :v] = bond[:, lo:hi]
        in_maps.append({
            "xT": xc, "heT": hc, "bond": bc,
            "w1t": w1t, "b1": bi, "b2": bm, "wmT": wmT,
        })
    return in_maps, n


def kernel(**inputs):
    global last_results
    from concourse.bass_utils import run_bass_kernel_spmd

    in_maps, n = marshal_inputs(inputs)
    nc = _get_nc()
    trace = os.environ.get("BASS_KERNEL_TRACE", "0") == "1"
    if trace:
        _ensure_ntff_hook()
    res = run_bass_kernel_spmd(nc, in_maps, core_ids=list(range(CORES)),
                               trace=trace)
    last_results = res
    hT = np.concatenate([r["h"] for r in res.results], axis=1)[:, :n]
    return hT.T.astype(np.float32, order="C")


# revision 17
# speedup vs baseline: 1.0800x; 1.0800x over previous
"""DMPNNConv kernel for 8 Trainium2 NeuronCores.

  h_n = relu([x ; h_e] @ W_i_w.T + W_i_b)          [N, D]
  m   = einsum('kn,nd->d', bond_n, h_n)            [D]
  h   = relu(h_n + m @ W_m_w.T + W_m_b)            [N, D]

Sharding: N (edge dim) split 8 ways; weights replicated; single [D]
all-gather (+local sum) of the message m between the two passes.

Design (vs the DRAM-scratch baseline):
  - host marshals x/h_e to feature-major fp8e4 (w1 pre-scaled x16,
    undone by the ACT scale) and bond to fp8, so the device streams
    large contiguous DMA runs with no SWDGE cast and no PE transposes.
  - z = W1.T @ [x;he] in ONE DoubleRow matmul (256-contract fp8).
  - h_n stays RESIDENT in SBUF ([128, 63488] bf16 = 124 KB/partition),
    eliminating the 32 MB/core DRAM scratch round-trip.
  - per 512-col tile: ACT relu+bias+scale -> resident hn; w broadcast
    = ones32.T@bond (PE); DVE prod = hn*wb; one batched DVE reduce per
    2048-col chunk accumulates the m partials.
  - [D] message all-gathered between passes; c = W_m m + b2 on PE.
  - pass 2: h = relu(hn + c) alternating ACT (per-partition bias) and
    DVE (stride-0 broadcast add + relu), bf16 out on two DMA queues;
    host upcasts/transposes the [D, N] output.
"""

import os
import sys

sys.path.insert(0, "/opt/trn_rl_repo")

import numpy as np

N, D, K = 500000, 128, 32
CORES = 8
T = 512                       # columns per compute tile (one PSUM bank)
NT = 124                      # tiles per core
N_SH = NT * T                 # 63488 padded tokens per core
N_PAD = CORES * N_SH          # 507904
CHUNK = 2048                  # columns per DMA chunk
NCH = N_SH // CHUNK           # 31
TPC = CHUNK // T              # tiles per chunk = 4
WSCALE = 16.0                 # fp8 weight pre-scale (undone in ACT)

USE_FP8 = os.environ.get("BASS_FP8", "1") == "1"

_cache = {}
last_results = None


def _build(nt=NT, chunk=CHUNK, fp8=None):
    import concourse.bass as bass  # noqa: F401
    import concourse.bacc as bacc
    import concourse.tile as tile
    import concourse.mybir as mybir

    n_sh = nt * T
    nch = n_sh // chunk
    tpc = chunk // T
    assert nch * chunk == n_sh
    if fp8 is None:
        fp8 = USE_FP8

    f32 = mybir.dt.float32
    bf16 = mybir.dt.bfloat16
    f8 = mybir.dt.float8e4
    in_dt = f8 if fp8 else bf16
    w1_fp8 = os.environ.get("BASS_W1DT", "bf16") == "fp8"
    w1_dt = f8 if (fp8 and w1_fp8) else bf16
    AF = mybir.ActivationFunctionType
    ALU = mybir.AluOpType
    red_mode = os.environ.get("BASS_RED", "reduce")

    nc = bacc.Bacc("TRN2", target_bir_lowering=False, debug=False,
                   num_devices=CORES)

    xT_d = nc.dram_tensor("xT", [D, n_sh], in_dt, kind="ExternalInput").ap()
    heT_d = nc.dram_tensor("heT", [D, n_sh], in_dt, kind="ExternalInput").ap()
    bond_d = nc.dram_tensor("bond", [K, n_sh], in_dt, kind="ExternalInput").ap()
    w1t_d = nc.dram_tensor("w1t", [D, 2 * D], w1_dt, kind="ExternalInput").ap()
    b1_d = nc.dram_tensor("b1", [D], f32, kind="ExternalInput").ap()
    b2_d = nc.dram_tensor("b2", [D], f32, kind="ExternalInput").ap()
    wmT_d = nc.dram_tensor("wmT", [D, D], f32, kind="ExternalInput").ap()
    h_d = nc.dram_tensor("h", [D, n_sh], bf16, kind="ExternalOutput").ap()

    with tile.TileContext(nc) as tc:
        import contextlib
        ctx = contextlib.ExitStack()
        with ctx:
            pers = ctx.enter_context(tc.tile_pool(name="pers", bufs=1))
            iox = ctx.enter_context(tc.tile_pool(name="iox", bufs=3))
            iob = ctx.enter_context(tc.tile_pool(name="iob", bufs=3))
            ioo = ctx.enter_context(tc.tile_pool(name="ioo", bufs=4))
            ps_z = ctx.enter_context(tc.tile_pool(name="ps_z", bufs=2, space="PSUM"))
            ps_w = ctx.enter_context(tc.tile_pool(name="ps_w", bufs=1, space="PSUM"))
            ps_c = ctx.enter_context(tc.tile_pool(name="ps_c", bufs=1, space="PSUM"))
            dram = ctx.enter_context(tc.tile_pool(name="dram", bufs=1, space="DRAM"))

            # ---- one-time setup -------------------------------------------
            # stationary weights: fp8 DoubleRow layout [j, 2, d] (x|he), or
            # bf16 [j, d] halves side by side.
            w1t = pers.tile([D, 2 * D], w1_dt)
            nc.sync.dma_start(w1t[:], w1t_d[:])
            b1_col = pers.tile([D, 1], f32)
            nc.sync.dma_start(b1_col[:, 0], b1_d[:])
            b2_col = pers.tile([D, 1], f32)
            nc.sync.dma_start(b2_col[:, 0], b2_d[:])
            wmt = pers.tile([D, D], f32)            # [d', d] = W_m.T
            nc.sync.dma_start(wmt[:], wmT_d[:])
            ones32 = pers.tile([K, D], in_dt)
            nc.gpsimd.memset(ones32[:], 1.0)

            hn_all = pers.tile([D, n_sh], bf16)     # resident h_n
            m_parts = pers.tile([D, nch], f32)
            junk = pers.tile([D, chunk], bf16)

            m_in = dram.tile([D], f32)
            use_ar = os.environ.get("BASS_CC", "ag") == "ar"
            if use_ar:
                m_out = dram.tile([D], f32, addr_space="Shared")
            else:
                m_gath = dram.tile([CORES * D], f32, addr_space="Shared")

            use_dr = fp8 and w1_fp8
            if use_dr:
                w1dr = w1t[:].rearrange("j (two d) -> j two d", two=2)
                dr_mode = mybir.MatmulPerfMode.DoubleRow
            zscale = 1.0 / WSCALE if use_dr else 1.0

            # ---- pass 1 ----------------------------------------------------
            for ch in range(nch):
                csl = slice(chunk * ch, chunk * (ch + 1))
                xc = iox.tile([D, 2, chunk], in_dt, tag="xhe")
                nc.sync.dma_start(xc[:, 0], xT_d[:, csl])
                nc.scalar.dma_start(xc[:, 1], heT_d[:, csl])
                bc = iob.tile([K, chunk], in_dt)
                nc.sync.dma_start(bc[:], bond_d[:, csl])

                prod = ioo.tile([D, chunk], bf16, tag="prod")
                # process the chunk in 1024-col pairs: stationary-major MM
                # order (back-to-back same-weight matmuls stream through the
                # PE without per-instruction reload bubbles), one 2-bank PSUM
                # pair per z/wb, single wide ACT/DVE ops.
                for p in range(tpc // 2):
                    lo = slice(1024 * p, 1024 * p + 512)
                    hi = slice(1024 * p + 512, 1024 * (p + 1))
                    g0 = chunk * ch + 1024 * p
                    gsl = slice(g0, g0 + 1024)
                    zp = ps_z.tile([D, 2 * T], f32, tag="z")
                    wbp = ps_w.tile([D, 2 * T], f32, tag="wb")
                    if use_dr:
                        nc.tensor.matmul(zp[:, 0:T], w1dr, xc[:, :, lo],
                                         start=True, stop=True,
                                         perf_mode=dr_mode,
                                         skip_group_check=True)
                        nc.tensor.matmul(zp[:, T:2 * T], w1dr, xc[:, :, hi],
                                         start=True, stop=True,
                                         perf_mode=dr_mode,
                                         skip_group_check=True)
                    else:
                        nc.tensor.matmul(zp[:, 0:T], w1t[:, 0:D],
                                         xc[:, 0, lo], start=True, stop=False,
                                         skip_group_check=True)
                        nc.tensor.matmul(zp[:, T:2 * T], w1t[:, 0:D],
                                         xc[:, 0, hi], start=True, stop=False,
                                         skip_group_check=True)
                        nc.tensor.matmul(zp[:, 0:T], w1t[:, D:2 * D],
                                         xc[:, 1, lo], start=False, stop=True,
                                         skip_group_check=True)
                        nc.tensor.matmul(zp[:, T:2 * T], w1t[:, D:2 * D],
                                         xc[:, 1, hi], start=False, stop=True,
                                         skip_group_check=True)
                    nc.tensor.matmul(wbp[:, 0:T], ones32[:], bc[:, lo],
                                     start=True, stop=True,
                                     skip_group_check=True)
                    nc.tensor.matmul(wbp[:, T:2 * T], ones32[:], bc[:, hi],
                                     start=True, stop=True,
                                     skip_group_check=True)

                    nc.scalar.activation(hn_all[:, gsl], zp[:], AF.Relu,
                                         bias=b1_col[:], scale=zscale)
                    nc.vector.tensor_tensor(prod[:, 1024 * p:1024 * (p + 1)],
                                            hn_all[:, gsl], wbp[:], ALU.mult)
                if red_mode == "ts":
                    nc.vector.tensor_scalar(
                        junk[:], prod[:], 1.0, 0.0, ALU.mult, ALU.add,
                        accum_out=m_parts[:, ch:ch + 1])
                else:
                    nc.vector.tensor_reduce(
                        m_parts[:, ch:ch + 1], prod[:],
                        mybir.AxisListType.X, ALU.add)

            # ---- m all-gather + local sum + c -----------------------------
            m_col = pers.tile([D, 1], f32)
            nc.vector.reduce_sum(m_col[:], m_parts[:], axis=mybir.AxisListType.X)
            nc.sync.dma_start(m_in[:], m_col[:, 0])
            m_sb = pers.tile([D, 1], f32)
            if use_ar:
                nc.gpsimd.collective_compute(
                    "AllReduce", ALU.add,
                    replica_groups=[list(range(CORES))],
                    ins=[m_in[:].opt()], outs=[m_out[:].opt()])
                nc.sync.dma_start(m_sb[:, 0], m_out[:])
            else:
                nc.gpsimd.collective_compute(
                    "AllGather", ALU.bypass,
                    replica_groups=[list(range(CORES))],
                    ins=[m_in[:].opt()], outs=[m_gath[:].opt()])
                m_g = pers.tile([D, CORES], f32)
                nc.sync.dma_start(m_g[:],
                                  m_gath[:].rearrange("(r p) -> p r", p=D))
                nc.vector.reduce_sum(m_sb[:], m_g[:],
                                     axis=mybir.AxisListType.X)

            c_ps = ps_c.tile([D, 1], f32, tag="c")
            nc.tensor.matmul(c_ps[:], wmt[:], m_sb[:], start=True, stop=True)
            c_col = pers.tile([D, 1], f32)
            nc.vector.tensor_tensor(c_col[:], c_ps[:], b2_col[:], ALU.add)
            c_bf = pers.tile([D, 1], bf16)
            nc.vector.tensor_copy(c_bf[:], c_col[:])

            # ---- pass 2: h = relu(hn + c), bf16 out -----------------------
            c_bc = c_bf[:].broadcast_to([D, chunk])
            for ch in range(nch):
                csl = slice(chunk * ch, chunk * (ch + 1))
                ob = ioo.tile([D, chunk], bf16, tag="ob")
                if ch % 2 == 0:
                    nc.scalar.activation(ob[:], hn_all[:, csl], AF.Relu,
                                         bias=c_col[:])
                else:
                    nc.vector.tensor_tensor(ob[:], hn_all[:, csl], c_bc,
                                            ALU.add)
                    nc.vector.tensor_relu(ob[:], ob[:])
                if ch % 2 == 0:
                    nc.sync.dma_start(h_d[:, csl], ob[:])
                else:
                    nc.scalar.dma_start(h_d[:, csl], ob[:])

    nc.compile()
    return nc


def _get_nc():
    if "nc" not in _cache:
        _cache["nc"] = _build()
    return _cache["nc"]


def _ensure_ntff_hook():
    """Register the axon NTFF profile hook if the image's antenv lacks it."""
    import types
    try:
        import antenv.axon_hooks  # noqa: F401
        return
    except ImportError:
        pass
    try:
        import antenv
        from trn_agent_boot.trn_boot import _ntff_profile_via_ctypes
        mod = types.ModuleType("antenv.axon_hooks")
        _h = {"hook": None}
        mod.set_axon_ntff_profile_hook = lambda h: _h.__setitem__("hook", h)
        mod.get_axon_ntff_profile_hook = lambda: _h["hook"]
        sys.modules["antenv.axon_hooks"] = mod
        antenv.axon_hooks = mod
        hook = _ntff_profile_via_ctypes("/opt/axon/libaxon_pjrt.so")
        if hook is not None:
            mod.set_axon_ntff_profile_hook(hook)
    except Exception:
        pass


def marshal_inputs(inputs, n_sh=N_SH, fp8=None):
    """Host-side marshalling: shard + feature-major layout + dtype cast."""
    import ml_dtypes
    if fp8 is None:
        fp8 = USE_FP8
    f8np = np.dtype(ml_dtypes.float8_e4m3)
    bfnp = np.dtype(ml_dtypes.bfloat16)
    in_np = f8np if fp8 else bfnp
    w1_fp8 = os.environ.get("BASS_W1DT", "bf16") == "fp8"
    w1_np = f8np if (fp8 and w1_fp8) else bfnp
    ws = WSCALE if (fp8 and w1_fp8) else 1.0

    x = np.asarray(inputs["x"], dtype=np.float32)
    he = np.asarray(inputs["h_e"], dtype=np.float32)
    bond = np.asarray(inputs["bond_n"], dtype=np.float32)
    wi = np.asarray(inputs["W_i_w"], dtype=np.float32)
    bi = np.ascontiguousarray(np.asarray(inputs["W_i_b"], dtype=np.float32))
    wm = np.asarray(inputs["W_m_w"], dtype=np.float32)
    bm = np.ascontiguousarray(np.asarray(inputs["W_m_b"], dtype=np.float32))

    n = x.shape[0]
    w1t = np.empty((D, 2 * D), w1_np)
    if fp8 and w1_fp8:
        # DoubleRow layout [j, 2, d]: slot 0 = x half, slot 1 = h_e half
        w1t3 = w1t.reshape(D, 2, D)
        w1t3[:, 0, :] = (wi[:, 0:D].T * ws).astype(w1_np)
        w1t3[:, 1, :] = (wi[:, D:2 * D].T * ws).astype(w1_np)
    else:
        w1t[:, 0:D] = wi[:, 0:D].T.astype(w1_np)
        w1t[:, D:2 * D] = wi[:, D:2 * D].T.astype(w1_np)
    wmT = np.ascontiguousarray(wm.T)

    xT, heT = x.T, he.T
    in_maps = []
    for c in range(CORES):
        lo = c * n_sh
        hi = min(n, lo + n_sh)
        v = max(0, hi - lo)
        xc = np.zeros((D, n_sh), in_np)
        hc = np.zeros((D, n_sh), in_np)
        bc = np.zeros((K, n_sh), in_np)
        if v > 0:
            xc[:, :v] = xT[:, lo:hi]
            hc[:, :v] = heT[:, lo:hi]
            bc[:, :v] = bond[:, lo:hi]
        in_maps.append({
            "xT": xc, "heT": hc, "bond": bc,
            "w1t": w1t, "b1": bi, "b2": bm, "wmT": wmT,
        })
    return in_maps, n


def kernel(**inputs):
    global last_results
    from concourse.bass_utils import run_bass_kernel_spmd

    in_maps, n = marshal_inputs(inputs)
    nc = _get_nc()
    trace = os.environ.get("BASS_KERNEL_TRACE", "0") == "1"
    if trace:
        _ensure_ntff_hook()
    res = run_bass_kernel_spmd(nc, in_maps, core_ids=list(range(CORES)),
                               trace=trace)
    last_results = res
    hT = np.concatenate([r["h"] for r in res.results], axis=1)[:, :n]
    return hT.T.astype(np.float32, order="C")
